# revision 29
# baseline (speedup 1.0000x reference)
"""Distributed Trainium2 kernel for causal RoPE multi-head attention.

Problem: y = OutProj(CausalSDPA(RoPE(QKV(x)))) with B=4, S=2048, D=2048,
H=16 heads, dh=128, fp32 reference.

Sharding (8 NeuronCores, one TRN2 chip):
  - QKV projection + RoPE + attention: tensor-parallel over heads.
    Core c owns global heads {2c, 2c+1} for all 4 batches.
  - A single 8-rank AllToAll redistributes the attention output from
    head-sharded to token-sharded: core c ends up with all 16 heads for
    its 1024 output tokens (batch c//2, sequence half c%2).
  - Output projection is then fully local; the host concatenates the 8
    [1024, 2048] shards into the [4, 2048, 2048] result.

Compute runs in bf16 on the TensorEngine (fp32 PSUM accumulation);
softmax statistics in fp32/fp32r.

Layout notes:
  - q/k are produced transposed ([feat, token], feat on partitions) so the
    scores matmul S^T = K^T_tile.T @ Q^T needs no transposes; v is produced
    token-major so P@V needs none either.
  - RoPE pairs are de-interleaved host-side (weight-row permutation): the
    kernel's q/k tiles hold the even dims of both heads in one 128-row tile
    (rows 0-63 head 2c, rows 64-127 head 2c+1) and the odd dims in another,
    making the rotation plain full-tile vector ops. Scores contract the two
    64-row halves with two accumulating K=64 matmuls (row-packed in the PE).
  - softmax: exp (no max subtraction needed; |scaled scores| < ~7), column
    sums via a DVE accumulator + one [128,1]-of-ones matmul, reciprocal
    broadcast back across partitions with a K=1 matmul.
"""

import os
import numpy as np

B, S, D = 4, 2048, 2048
H, DH = 16, 128
SCALE = 1.0 / float(np.sqrt(DH))
NCORES = 8

_CACHE = {}

LAST_RESULT = None  # BassKernelResults of most recent run (for test harness)


def _build_nc():
    import concourse.bacc as bacc
    import concourse.tile as tile
    from concourse import mybir
    from contextlib import ExitStack

    BF = mybir.dt.bfloat16
    F32 = mybir.dt.float32
    F32R = mybir.dt.float32r

    nc = bacc.Bacc(None)
    with tile.TileContext(nc) as tc, ExitStack() as ctx:
        dram = ctx.enter_context(tc.tile_pool(name="dram", bufs=1, space="DRAM"))
        xT_e = dram.tile([B, 4, 128, 16, 512], BF, kind="ExternalInput", name="xT", uniquify=False)
        wqkT_e = dram.tile([128, 16, 512], BF, kind="ExternalInput", name="wqkT", uniquify=False)
        wvT_e = dram.tile([128, 16, 256], BF, kind="ExternalInput", name="wvT", uniquify=False)
        outwT_e = dram.tile([8, 128, 16, 256], BF, kind="ExternalInput", name="outwT", uniquify=False)
        cs_e = dram.tile([128, S], BF, kind="ExternalInput", name="cs", uniquify=False)
        sn_e = dram.tile([128, S], BF, kind="ExternalInput", name="sn", uniquify=False)
        masks_e = dram.tile([128, 4, 512], BF, kind="ExternalInput", name="masks", uniquify=False)
        out_e = dram.tile([1024, D], F32, kind="ExternalOutput", name="out", uniquify=False)
        a2a_ins = [dram.tile([8, 2, 128, 256], BF, name=f"a2a_in{i}") for i in range(B)]
        a2a_outs = [dram.tile([8, 2, 128, 256], BF, name=f"a2a_out{i}") for i in range(B)]

        # ---- SBUF pools ----
        big = ctx.enter_context(tc.tile_pool(name="big", bufs=1))        # x (64KB/p) / y_res
        rot = ctx.enter_context(tc.tile_pool(name="rot", bufs=12))        # rotated q/k, 4KB/p each
        vpool = ctx.enter_context(tc.tile_pool(name="vpool", bufs=1))    # v per batch, 8KB/p
        wpool = ctx.enter_context(tc.tile_pool(name="wpool", bufs=1))    # wqk (16KB/p)
        wvp = ctx.enter_context(tc.tile_pool(name="wvp", bufs=1))        # wv (8KB/p)
        csp = ctx.enter_context(tc.tile_pool(name="csp", bufs=1))        # cos/sin (8KB/p)
        mkp = ctx.enter_context(tc.tile_pool(name="mkp", bufs=1))        # masks (4KB/p)
        mtp = ctx.enter_context(tc.tile_pool(name="mtp", bufs=8))        # rope temps 1KB/p
        ep = ctx.enter_context(tc.tile_pool(name="ep", bufs=4))          # exp tiles 1KB/p
        accp = ctx.enter_context(tc.tile_pool(name="accp", bufs=2))      # colsum acc 2KB/p
        rbp = ctx.enter_context(tc.tile_pool(name="rbp", bufs=1))        # recip bcast 2KB/p
        ysp = ctx.enter_context(tc.tile_pool(name="ysp", bufs=3))        # y out tiles 1KB/p
        onep = ctx.enter_context(tc.tile_pool(name="onep", bufs=1))
        owp = ctx.enter_context(tc.tile_pool(name="owp", bufs=3))        # outw stream 16KB/p
        oep = ctx.enter_context(tc.tile_pool(name="oep", bufs=4))        # out evict 2KB/p

        psA = ctx.enter_context(tc.tile_pool(name="psA", bufs=5, space="PSUM"))
        psY = ctx.enter_context(tc.tile_pool(name="psY", bufs=2, space="PSUM"))
        psR = ctx.enter_context(tc.tile_pool(name="psR", bufs=1, space="PSUM"))

        # ---- constants / weights ----
        wqk_sb = wpool.tile([128, 16, 512], BF)
        for dc in range(4):
            nc.sync.dma_start(out=wqk_sb[:, 4 * dc:4 * dc + 4, :],
                              in_=wqkT_e[:, 4 * dc:4 * dc + 4, :])
        wv_sb = wvp.tile([128, 16, 256], BF)
        nc.sync.dma_start(out=wv_sb[:], in_=wvT_e[:])
        cs_sb = csp.tile([128, S], BF)
        nc.scalar.dma_start(out=cs_sb[:], in_=cs_e[:])
        sn_sb = csp.tile([128, S], BF)
        nc.scalar.dma_start(out=sn_sb[:], in_=sn_e[:])
        mk_sb = mkp.tile([128, 4, 512], BF)
        nc.scalar.dma_start(out=mk_sb[:], in_=masks_e[:])
        ones_full = onep.tile([128, 128], BF)
        nc.vector.memset(ones_full[:], 1.0)

        # ---------- emission helpers (interleaved software pipeline) ----------
        def emit_x_load(b):
            x_sb = big.tile([128, 4, 16, 512], BF, tag="bigbuf", name=f"x_sb_{b}")
            for tb4 in range(4):
                if b == 0:
                    nc.gpsimd.dma_start(out=x_sb[:, tb4, 0:8], in_=xT_e[b, tb4, :, 0:8])
                    nc.gpsimd.dma_start(out=x_sb[:, tb4, 8:16], in_=xT_e[b, tb4, :, 8:16])
                else:
                    nc.gpsimd.dma_start(out=x_sb[:, tb4], in_=xT_e[b, tb4])
            return x_sb

        def make_qkv_groups(b, x_sb, st):
            """Closure list: 8 qk (pair, tb) chain groups + merges + 4 v groups."""
            groups = []
            pair_tiles = {}

            def qk_sub(pair, tb):
                if tb == 0:
                    pair_tiles[pair] = (
                        rot.tile([128, S], BF, tag="rot", name=f"rA_{b}_{pair}"),
                        rot.tile([128, S], BF, tag="rot", name=f"rB_{b}_{pair}"))
                rA, rB = pair_tiles[pair]
                tsl = slice(tb * 512, tb * 512 + 512)
                psa = psA.tile([128, 512], F32, tag="ps", name=f"psqa_{b}_{pair}_{tb}")
                for d in range(16):
                    nc.tensor.matmul(
                        psa[:], wqk_sb[:, d, pair * 256:pair * 256 + 128],
                        x_sb[:, tb, d, :], start=(d == 0), stop=(d == 15))
                psb = psA.tile([128, 512], F32, tag="ps", name=f"psqb_{b}_{pair}_{tb}")
                for d in range(16):
                    nc.tensor.matmul(
                        psb[:], wqk_sb[:, d, pair * 256 + 128:pair * 256 + 256],
                        x_sb[:, tb, d, :], start=(d == 0), stop=(d == 15))
                # rope: rA = A*cos - B*sin ; rB = A*sin + B*cos
                ra_ = mtp.tile([128, 512], BF, tag="mt", name=f"ra_{b}_{pair}_{tb}")
                rb_ = mtp.tile([128, 512], BF, tag="mt", name=f"rb_{b}_{pair}_{tb}")
                nc.scalar.copy(ra_[:], psa[:])
                nc.scalar.copy(rb_[:], psb[:])
                m1 = mtp.tile([128, 512], BF, tag="mt", name=f"m1_{b}_{pair}_{tb}")
                m2 = mtp.tile([128, 512], BF, tag="mt", name=f"m2_{b}_{pair}_{tb}")
                m3 = mtp.tile([128, 512], BF, tag="mt", name=f"m3_{b}_{pair}_{tb}")
                m4 = mtp.tile([128, 512], BF, tag="mt", name=f"m4_{b}_{pair}_{tb}")
                nc.vector.tensor_mul(m1[:], ra_[:], cs_sb[:, tsl])
                nc.vector.tensor_mul(m2[:], rb_[:], sn_sb[:, tsl])
                nc.vector.tensor_mul(m3[:], ra_[:], sn_sb[:, tsl])
                nc.vector.tensor_mul(m4[:], rb_[:], cs_sb[:, tsl])
                nc.vector.tensor_sub(rA[:, tsl], m1[:], m2[:])
                nc.vector.tensor_add(rB[:, tsl], m3[:], m4[:])
                if tb == 3:
                    # merge halves into per-head [128, S] tiles
                    h0 = rot.tile([128, S], BF, tag="rot", name=f"h0_{b}_{pair}")
                    h1 = rot.tile([128, S], BF, tag="rot", name=f"h1_{b}_{pair}")
                    nc.sync.dma_start(out=h0[0:64, :], in_=rA[0:64, :])
                    nc.sync.dma_start(out=h0[64:128, :], in_=rB[0:64, :])
                    nc.sync.dma_start(out=h1[0:64, :], in_=rA[64:128, :])
                    nc.sync.dma_start(out=h1[64:128, :], in_=rB[64:128, :])
                    st["rots"].append((h0, h1))

            for pair in range(2):
                for tb in range(4):
                    groups.append(lambda pair=pair, tb=tb: qk_sub(pair, tb))

            def v_sub(quarter):
                if quarter == 0:
                    st["v"] = vpool.tile([128, 16, 256], BF, tag="v", name=f"v_sb_{b}")
                v_sb = st["v"]
                for tt in range(quarter * 4, quarter * 4 + 4):
                    psv = psA.tile([128, 256], F32, tag="ps", name=f"psv_{b}_{tt}")
                    for d in range(16):
                        nc.tensor.matmul(
                            psv[:], x_sb[:, tt // 4, d, (tt % 4) * 128:(tt % 4) * 128 + 128],
                            wv_sb[:, d, :], start=(d == 0), stop=(d == 15))
                    nc.scalar.copy(v_sb[:, tt, :], psv[:])

            vgroups = [lambda quarter=quarter: v_sub(quarter) for quarter in range(4)]
            return groups, vgroups

        def make_attn_units(b, st):
            qh, kh = st["rots"][0], st["rots"][1]
            fstate = {"pend": None}

            def finalize(acc, yps, e, qb):
                # all-ones lhsT: out[m,n] = sum_k acc[k,n] for every m —
                # softmax denominator summed AND partition-broadcast in one matmul
                rps = psR.tile([128, 512], F32, tag="pr", name=f"rps_{b}_{e}_{qb}")
                nc.tensor.matmul(rps[:], ones_full[:], acc[:], start=True, stop=True)
                rb = rbp.tile([128, 512], F32, tag="rb", name=f"rb_{b}_{e}_{qb}")
                nc.vector.reciprocal_approx_fast(out=rb[:], in_=rps[:])
                ysb = ysp.tile([128, 512], BF, tag="ys", name=f"ysb_{b}_{e}_{qb}")
                nc.vector.tensor_mul(ysb[:], yps[:], rb[:])
                nc.sync.dma_start(out=a2a_ins[b][2 * qb, e, :, :], in_=ysb[:, 0:256])
                nc.sync.dma_start(out=a2a_ins[b][2 * qb + 1, e, :, :], in_=ysb[:, 256:512])

            def unit(qb, e):
                v_sb = st["v"]
                q_he, k_he = qh[e], kh[e]
                qsl = slice(qb * 512, qb * 512 + 512)
                nkt = 4 * qb + 4
                acc = accp.tile([128, 512], BF, tag="acc", name=f"acc_{b}_{e}_{qb}")
                yps = psY.tile([128, 512], F32, tag="py", name=f"yps_{b}_{e}_{qb}")
                for kt in range(nkt):
                    # diagonal-region units: queries below kt*128 are fully
                    # masked -- narrow all ops to the valid column range
                    r = kt - 4 * qb
                    off = r * 128 if r > 0 else 0
                    w = 512 - off
                    sps = psA.tile([128, 512], F32, tag="ps", name=f"sps_{b}_{e}_{qb}_{kt}")
                    ksl = slice(kt * 128, kt * 128 + 128)
                    nc.tensor.matmul(sps[:, 0:w], k_he[:, ksl],
                                     q_he[:, qb * 512 + off:qb * 512 + 512],
                                     start=True, stop=True)
                    et = ep.tile([128, 512], BF, tag="et", name=f"et_{b}_{e}_{qb}_{kt}")
                    if off > 0:
                        nc.vector.memset(et[:, 0:off], 0.0)
                    nc.scalar.activation(et[:, off:512], sps[:, 0:w],
                                         mybir.ActivationFunctionType.Exp, scale=SCALE)
                    if r >= 0:
                        nc.vector.tensor_mul(et[:, off:512], et[:, off:512],
                                             mk_sb[:, r, off:512])
                    if kt == 0:
                        nc.vector.tensor_copy(acc[:], et[:])
                    else:
                        nc.vector.tensor_add(acc[:, off:512], acc[:, off:512],
                                             et[:, off:512])
                    nc.tensor.matmul(yps[:], v_sb[:, kt, e * 128:e * 128 + 128],
                                     et[:], start=(kt == 0), stop=(kt == nkt - 1))
                    if kt == 1 and fstate["pend"] is not None:
                        finalize(*fstate["pend"])
                        fstate["pend"] = None
                if fstate["pend"] is not None:
                    finalize(*fstate["pend"])
                fstate["pend"] = (acc, yps, e, qb)

            units = [lambda qb=qb, e=e: unit(qb, e) for qb in range(4) for e in range(2)]

            def tail():
                finalize(*fstate["pend"])
                nc.gpsimd.collective_compute(
                    "AllToAll", mybir.AluOpType.bypass,
                    ins=[a2a_ins[b][:]], outs=[a2a_outs[b][:]],
                    replica_groups=[list(range(NCORES))],
                )
            return units, tail

        def emit_yres_load(y_res, b):
            for j in range(8):
                for e in range(2):
                    nc.gpsimd.dma_start(out=y_res[:, 2 * j + e, b * 256:b * 256 + 256],
                                        in_=a2a_outs[b][j, e])

        def outproj_db(y_res, tag, db, i_list, eng=None):
            owt = owp.tile([128, 16, 256], BF, tag="ow", name=f"owt_{tag}_{db}")
            (eng or nc.sync).dma_start(out=owt[:], in_=outwT_e[db])
            dsl = slice(db * 256, db * 256 + 256)
            for i in i_list:
                pso = psA.tile([128, 256], F32, tag="ps", name=f"pso_{tag}_{db}_{i}")
                for ft in range(16):
                    nc.tensor.matmul(pso[:], y_res[:, ft, i * 128:i * 128 + 128],
                                     owt[:, ft, :], start=(ft == 0), stop=(ft == 15))
                oev = oep.tile([128, 256], F32, tag="oe", name=f"oev_{tag}_{db}_{i}")
                nc.vector.tensor_copy(oev[:], pso[:])
                nc.sync.dma_start(out=out_e[i * 128:i * 128 + 128, dsl], in_=oev[:])

        def interleave(units, groups, tail):
            """alternate unit/group; leftovers appended; tail last"""
            ui, gi = 0, 0
            while ui < len(units) or gi < len(groups):
                if ui < len(units):
                    units[ui](); ui += 1
                if gi < len(groups):
                    groups[gi](); gi += 1
            tail()

        # ---------- pipeline (sequential per batch) ----------
        x_sb = emit_x_load(0)
        st = {"rots": [], "v": None}
        qk0, v0 = make_qkv_groups(0, x_sb, st)
        for g in qk0:
            g()
        vgroups = v0
        y_res = None
        for b in range(B):
            vgroups[0]()  # v quarter 0 before the first unit
            units, tail = make_attn_units(b, st)
            if b == B - 1:
                # last batch: emit all v quarters up front (frees the x slot so
                # y_res can allocate), then interleave batch-0/1 out-proj tiles
                # into the attention units (their A2As completed batches ago)
                for q in (1, 2, 3):
                    vgroups[q]()
                y_res = big.tile([128, 16, 1024], BF, tag="bigbuf", name="y_res")
                emit_yres_load(y_res, 0)
                emit_yres_load(y_res, 1)
                op_groups = [lambda db=db: outproj_db(y_res, "a", db, [0, 1, 2, 3],
                                                      eng=nc.gpsimd)
                             for db in range(8)]
            else:
                op_groups = []
            gi = 0
            for i, u in enumerate(units):
                u()
                # unit (qb, e): after both units of qb, emit v quarter qb+1
                if b < B - 1 and i in (1, 3, 5) and (i // 2 + 1) < 4:
                    vgroups[i // 2 + 1]()
                if op_groups and gi < len(op_groups):
                    op_groups[gi](); gi += 1
            while gi < len(op_groups):
                op_groups[gi](); gi += 1
            tail()
            if b < B - 1:
                xn = emit_x_load(b + 1)
                st_next = {"rots": [], "v": None}
                qkn, vn = make_qkv_groups(b + 1, xn, st_next)
                for g in qkn:
                    g()
                vgroups = vn
                st = st_next

        # ---- remaining output projection (batch-3 tiles last, hiding the final A2A) ----
        emit_yres_load(y_res, 2)
        for db in range(8):
            outproj_db(y_res, "b", db, [4, 5])
        emit_yres_load(y_res, 3)
        for db in range(8):
            outproj_db(y_res, "c", db, [6, 7])

    nc.compile()
    return nc


def _host_prep(x, qkv_w, out_w):
    """Build the per-core input maps (bf16, pre-transposed/permuted)."""
    import ml_dtypes
    bf16 = ml_dtypes.bfloat16

    # x_pre[b, tb, p, d, s] = x[b, tb*512+s, d*128+p]
    xT = np.ascontiguousarray(
        x.reshape(B, 4, 512, 16, 128).transpose(0, 1, 4, 3, 2)).astype(bf16)
    # outw_pre[db, p, ft, n] = out_w.T[ft*128+p, db*256+n]
    outwT = np.ascontiguousarray(
        out_w.T.reshape(16, 128, 8, 256).transpose(2, 1, 0, 3)).astype(bf16)

    even = np.arange(0, DH, 2)
    odd = np.arange(1, DH, 2)
    freqs = 1.0 / (10000.0 ** (np.arange(0, DH, 2, dtype=np.float64) / DH))
    ang = np.arange(S, dtype=np.float64)[None, :] * freqs[:, None]   # [64, S]
    cs = np.concatenate([np.cos(ang), np.cos(ang)], 0).astype(bf16)  # [128, S]
    sn = np.concatenate([np.sin(ang), np.sin(ang)], 0).astype(bf16)

    masks = np.zeros((4, 128, 512), np.float32)
    for r in range(4):
        for t in range(128):
            masks[r, t, r * 128 + t:] = 1.0
    masks = np.ascontiguousarray(masks.transpose(1, 0, 2)).astype(bf16)  # [128, 4, 512]

    in_maps = []
    for c in range(NCORES):
        h0, h1 = 2 * c, 2 * c + 1
        qA = np.concatenate([h0 * DH + even, h1 * DH + even])
        qB = np.concatenate([h0 * DH + odd, h1 * DH + odd])
        rows_qk = np.concatenate([qA, qB, 2048 + qA, 2048 + qB])
        # wqk_pre[p, d, f] = qkv_w[rows_qk[f], d*128+p]
        wqkT = np.ascontiguousarray(
            qkv_w[rows_qk].T.reshape(16, 128, 512).transpose(1, 0, 2)).astype(bf16)
        wvT = np.ascontiguousarray(
            qkv_w[4096 + h0 * DH: 4096 + (h1 + 1) * DH].T.reshape(16, 128, 256)
            .transpose(1, 0, 2)).astype(bf16)
        in_maps.append({
            "xT": xT, "wqkT": wqkT, "wvT": wvT, "outwT": outwT,
            "cs": cs, "sn": sn, "masks": masks,
        })
    return in_maps


def _ensure_profile_hook():
    """The agent image's antenv lacks axon_hooks; recreate it so that
    run_bass_kernel_spmd(trace=True) (or BASS_TRACE=1) does not crash."""
    import sys, types
    try:
        import antenv.axon_hooks  # noqa
        return
    except ImportError:
        pass
    try:
        from trn_agent_boot.trn_boot import _ntff_profile_via_ctypes
        hook = _ntff_profile_via_ctypes("/opt/axon/libaxon_pjrt.so")
    except Exception:
        hook = None
    mod = types.ModuleType("antenv.axon_hooks")
    mod.get_axon_ntff_profile_hook = lambda: hook

    def set_axon_ntff_profile_hook(h):
        mod.get_axon_ntff_profile_hook = lambda: h

    mod.set_axon_ntff_profile_hook = set_axon_ntff_profile_hook
    sys.modules["antenv.axon_hooks"] = mod
    try:
        import antenv
        antenv.axon_hooks = mod
    except ImportError:
        pass


def kernel(x, qkv_w, qkv_b, out_w, out_b):
    global LAST_RESULT
    from concourse.bass_utils import run_bass_kernel_spmd
    _ensure_profile_hook()

    if "nc" not in _CACHE:
        _CACHE["nc"] = _build_nc()
    nc = _CACHE["nc"]

    in_maps = _host_prep(np.asarray(x, np.float32), np.asarray(qkv_w, np.float32),
                         np.asarray(out_w, np.float32))
    trace = bool(os.environ.get("BASS_KERNEL_TRACE"))
    r = run_bass_kernel_spmd(nc, in_maps, list(range(NCORES)), trace=trace)
    LAST_RESULT = r

    out = np.empty((B, S, D), np.float32)
    for c in range(NCORES):
        shard = r.results[c]["out"]
        for b in range(B):
            out[b, c * 256:(c + 1) * 256, :] = shard[b * 256:(b + 1) * 256]
    return out


# revision 30
# speedup vs baseline: 1.0372x; 1.0372x over previous
"""Distributed Trainium2 kernel for causal RoPE multi-head attention.

Problem: y = OutProj(CausalSDPA(RoPE(QKV(x)))) with B=4, S=2048, D=2048,
H=16 heads, dh=128, fp32 reference.

Sharding (8 NeuronCores, one TRN2 chip):
  - QKV projection + RoPE + attention: tensor-parallel over heads.
    Core c owns global heads {2c, 2c+1} for all 4 batches.
  - A single 8-rank AllToAll redistributes the attention output from
    head-sharded to token-sharded: core c ends up with all 16 heads for
    its 1024 output tokens (batch c//2, sequence half c%2).
  - Output projection is then fully local; the host concatenates the 8
    [1024, 2048] shards into the [4, 2048, 2048] result.

Compute runs in bf16 on the TensorEngine (fp32 PSUM accumulation);
softmax statistics in fp32/fp32r.

Layout notes:
  - q/k are produced transposed ([feat, token], feat on partitions) so the
    scores matmul S^T = K^T_tile.T @ Q^T needs no transposes; v is produced
    token-major so P@V needs none either.
  - RoPE pairs are de-interleaved host-side (weight-row permutation): the
    kernel's q/k tiles hold the even dims of both heads in one 128-row tile
    (rows 0-63 head 2c, rows 64-127 head 2c+1) and the odd dims in another,
    making the rotation plain full-tile vector ops. Scores contract the two
    64-row halves with two accumulating K=64 matmuls (row-packed in the PE).
  - softmax: exp (no max subtraction needed; |scaled scores| < ~7), column
    sums via a DVE accumulator + one [128,1]-of-ones matmul, reciprocal
    broadcast back across partitions with a K=1 matmul.
"""

import os
import numpy as np

B, S, D = 4, 2048, 2048
H, DH = 16, 128
SCALE = 1.0 / float(np.sqrt(DH))
NCORES = 8

_CACHE = {}

LAST_RESULT = None  # BassKernelResults of most recent run (for test harness)


def _build_nc():
    import concourse.bacc as bacc
    import concourse.tile as tile
    from concourse import mybir
    from contextlib import ExitStack

    BF = mybir.dt.bfloat16
    F32 = mybir.dt.float32
    F32R = mybir.dt.float32r

    nc = bacc.Bacc(None)
    with tile.TileContext(nc) as tc, ExitStack() as ctx:
        dram = ctx.enter_context(tc.tile_pool(name="dram", bufs=1, space="DRAM"))
        xT_e = dram.tile([B, 4, 128, 16, 512], BF, kind="ExternalInput", name="xT", uniquify=False)
        wqkT_e = dram.tile([128, 16, 512], BF, kind="ExternalInput", name="wqkT", uniquify=False)
        wvT_e = dram.tile([128, 16, 256], BF, kind="ExternalInput", name="wvT", uniquify=False)
        outwT_e = dram.tile([8, 128, 16, 256], BF, kind="ExternalInput", name="outwT", uniquify=False)
        cs_e = dram.tile([128, S], BF, kind="ExternalInput", name="cs", uniquify=False)
        sn_e = dram.tile([128, S], BF, kind="ExternalInput", name="sn", uniquify=False)
        masks_e = dram.tile([128, 4, 512], BF, kind="ExternalInput", name="masks", uniquify=False)
        out_e = dram.tile([1024, D], F32, kind="ExternalOutput", name="out", uniquify=False)
        a2a_ins = [dram.tile([8, 2, 128, 256], BF, name=f"a2a_in{i}") for i in range(B)]
        a2a_outs = [dram.tile([8, 2, 128, 256], BF, name=f"a2a_out{i}") for i in range(B)]

        # ---- SBUF pools ----
        big = ctx.enter_context(tc.tile_pool(name="big", bufs=1))        # x (64KB/p) / y_res
        rot = ctx.enter_context(tc.tile_pool(name="rot", bufs=12))        # rotated q/k, 4KB/p each
        vpool = ctx.enter_context(tc.tile_pool(name="vpool", bufs=1))    # v per batch, 8KB/p
        wpool = ctx.enter_context(tc.tile_pool(name="wpool", bufs=1))    # wqk (16KB/p)
        wvp = ctx.enter_context(tc.tile_pool(name="wvp", bufs=1))        # wv (8KB/p)
        csp = ctx.enter_context(tc.tile_pool(name="csp", bufs=1))        # cos/sin (8KB/p)
        mkp = ctx.enter_context(tc.tile_pool(name="mkp", bufs=1))        # masks (4KB/p)
        mtp = ctx.enter_context(tc.tile_pool(name="mtp", bufs=8))        # rope temps 1KB/p
        ep = ctx.enter_context(tc.tile_pool(name="ep", bufs=4))          # exp tiles 1KB/p
        accp = ctx.enter_context(tc.tile_pool(name="accp", bufs=2))      # colsum acc 2KB/p
        rbp = ctx.enter_context(tc.tile_pool(name="rbp", bufs=1))        # recip bcast 2KB/p
        ysp = ctx.enter_context(tc.tile_pool(name="ysp", bufs=3))        # y out tiles 1KB/p
        onep = ctx.enter_context(tc.tile_pool(name="onep", bufs=1))
        owp = ctx.enter_context(tc.tile_pool(name="owp", bufs=3))        # outw stream 16KB/p
        oep = ctx.enter_context(tc.tile_pool(name="oep", bufs=4))        # out evict 2KB/p

        psA = ctx.enter_context(tc.tile_pool(name="psA", bufs=5, space="PSUM"))
        psY = ctx.enter_context(tc.tile_pool(name="psY", bufs=2, space="PSUM"))
        psR = ctx.enter_context(tc.tile_pool(name="psR", bufs=1, space="PSUM"))

        # ---- constants / weights ----
        wqk_sb = wpool.tile([128, 16, 512], BF)
        for dc in range(4):
            nc.sync.dma_start(out=wqk_sb[:, 4 * dc:4 * dc + 4, :],
                              in_=wqkT_e[:, 4 * dc:4 * dc + 4, :])
        wv_sb = wvp.tile([128, 16, 256], BF)
        nc.sync.dma_start(out=wv_sb[:], in_=wvT_e[:])
        cs_sb = csp.tile([128, S], BF)
        nc.scalar.dma_start(out=cs_sb[:], in_=cs_e[:])
        sn_sb = csp.tile([128, S], BF)
        nc.scalar.dma_start(out=sn_sb[:], in_=sn_e[:])
        mk_sb = mkp.tile([128, 4, 512], BF)
        nc.scalar.dma_start(out=mk_sb[:], in_=masks_e[:])
        ones_full = onep.tile([128, 128], BF)
        nc.vector.memset(ones_full[:], 1.0)

        # ---------- emission helpers (interleaved software pipeline) ----------
        def emit_x_load(b):
            x_sb = big.tile([128, 4, 16, 512], BF, tag="bigbuf", name=f"x_sb_{b}")
            for tb4 in range(4):
                if b == 0:
                    nc.gpsimd.dma_start(out=x_sb[:, tb4, 0:8], in_=xT_e[b, tb4, :, 0:8])
                    nc.gpsimd.dma_start(out=x_sb[:, tb4, 8:16], in_=xT_e[b, tb4, :, 8:16])
                else:
                    nc.gpsimd.dma_start(out=x_sb[:, tb4], in_=xT_e[b, tb4])
            return x_sb

        def make_qkv_groups(b, x_sb, st):
            """Closure list: 8 qk (pair, tb) chain groups + merges + 4 v groups."""
            groups = []
            pair_tiles = {}

            def qk_sub(pair, tb):
                if tb == 0:
                    pair_tiles[pair] = (
                        rot.tile([128, S], BF, tag="rot", name=f"rA_{b}_{pair}"),
                        rot.tile([128, S], BF, tag="rot", name=f"rB_{b}_{pair}"))
                rA, rB = pair_tiles[pair]
                tsl = slice(tb * 512, tb * 512 + 512)
                psa = psA.tile([128, 512], F32, tag="ps", name=f"psqa_{b}_{pair}_{tb}")
                for d in range(16):
                    nc.tensor.matmul(
                        psa[:], wqk_sb[:, d, pair * 256:pair * 256 + 128],
                        x_sb[:, tb, d, :], start=(d == 0), stop=(d == 15))
                psb = psA.tile([128, 512], F32, tag="ps", name=f"psqb_{b}_{pair}_{tb}")
                for d in range(16):
                    nc.tensor.matmul(
                        psb[:], wqk_sb[:, d, pair * 256 + 128:pair * 256 + 256],
                        x_sb[:, tb, d, :], start=(d == 0), stop=(d == 15))
                # rope: rA = A*cos - B*sin ; rB = A*sin + B*cos
                ra_ = mtp.tile([128, 512], BF, tag="mt", name=f"ra_{b}_{pair}_{tb}")
                rb_ = mtp.tile([128, 512], BF, tag="mt", name=f"rb_{b}_{pair}_{tb}")
                nc.scalar.copy(ra_[:], psa[:])
                nc.scalar.copy(rb_[:], psb[:])
                m1 = mtp.tile([128, 512], BF, tag="mt", name=f"m1_{b}_{pair}_{tb}")
                m2 = mtp.tile([128, 512], BF, tag="mt", name=f"m2_{b}_{pair}_{tb}")
                m3 = mtp.tile([128, 512], BF, tag="mt", name=f"m3_{b}_{pair}_{tb}")
                m4 = mtp.tile([128, 512], BF, tag="mt", name=f"m4_{b}_{pair}_{tb}")
                nc.vector.tensor_mul(m1[:], ra_[:], cs_sb[:, tsl])
                nc.vector.tensor_mul(m2[:], rb_[:], sn_sb[:, tsl])
                nc.vector.tensor_mul(m3[:], ra_[:], sn_sb[:, tsl])
                nc.vector.tensor_mul(m4[:], rb_[:], cs_sb[:, tsl])
                nc.vector.tensor_sub(rA[:, tsl], m1[:], m2[:])
                nc.vector.tensor_add(rB[:, tsl], m3[:], m4[:])
                if tb == 3:
                    # merge halves into per-head [128, S] tiles
                    h0 = rot.tile([128, S], BF, tag="rot", name=f"h0_{b}_{pair}")
                    h1 = rot.tile([128, S], BF, tag="rot", name=f"h1_{b}_{pair}")
                    nc.sync.dma_start(out=h0[0:64, :], in_=rA[0:64, :])
                    nc.sync.dma_start(out=h0[64:128, :], in_=rB[0:64, :])
                    nc.sync.dma_start(out=h1[0:64, :], in_=rA[64:128, :])
                    nc.sync.dma_start(out=h1[64:128, :], in_=rB[64:128, :])
                    st["rots"].append((h0, h1))

            for pair in range(2):
                for tb in range(4):
                    groups.append(lambda pair=pair, tb=tb: qk_sub(pair, tb))

            def v_sub(quarter):
                if quarter == 0:
                    st["v"] = vpool.tile([128, 16, 256], BF, tag="v", name=f"v_sb_{b}")
                v_sb = st["v"]
                for tt in range(quarter * 4, quarter * 4 + 4):
                    psv = psA.tile([128, 256], F32, tag="ps", name=f"psv_{b}_{tt}")
                    for d in range(16):
                        nc.tensor.matmul(
                            psv[:], x_sb[:, tt // 4, d, (tt % 4) * 128:(tt % 4) * 128 + 128],
                            wv_sb[:, d, :], start=(d == 0), stop=(d == 15))
                    nc.scalar.copy(v_sb[:, tt, :], psv[:])

            vgroups = [lambda quarter=quarter: v_sub(quarter) for quarter in range(4)]
            return groups, vgroups

        def make_attn_units(b, st):
            qh, kh = st["rots"][0], st["rots"][1]
            fstate = {"pend": None}

            def finalize(acc, yps, e, qb):
                # all-ones lhsT: out[m,n] = sum_k acc[k,n] for every m —
                # softmax denominator summed AND partition-broadcast in one matmul
                rps = psR.tile([128, 512], F32, tag="pr", name=f"rps_{b}_{e}_{qb}")
                nc.tensor.matmul(rps[:], ones_full[:], acc[:], start=True, stop=True)
                rb = rbp.tile([128, 512], F32, tag="rb", name=f"rb_{b}_{e}_{qb}")
                nc.vector.reciprocal_approx_fast(out=rb[:], in_=rps[:])
                ysb = ysp.tile([128, 512], BF, tag="ys", name=f"ysb_{b}_{e}_{qb}")
                nc.vector.tensor_mul(ysb[:], yps[:], rb[:])
                nc.sync.dma_start(out=a2a_ins[b][2 * qb, e, :, :], in_=ysb[:, 0:256])
                nc.sync.dma_start(out=a2a_ins[b][2 * qb + 1, e, :, :], in_=ysb[:, 256:512])

            def unit(qb, e):
                v_sb = st["v"]
                q_he, k_he = qh[e], kh[e]
                qsl = slice(qb * 512, qb * 512 + 512)
                nkt = 4 * qb + 4
                acc = accp.tile([128, 512], BF, tag="acc", name=f"acc_{b}_{e}_{qb}")
                yps = psY.tile([128, 512], F32, tag="py", name=f"yps_{b}_{e}_{qb}")
                for kt in range(nkt):
                    # diagonal-region units: queries below kt*128 are fully
                    # masked -- narrow all ops to the valid column range
                    r = kt - 4 * qb
                    off = r * 128 if r > 0 else 0
                    w = 512 - off
                    sps = psA.tile([128, 512], F32, tag="ps", name=f"sps_{b}_{e}_{qb}_{kt}")
                    ksl = slice(kt * 128, kt * 128 + 128)
                    nc.tensor.matmul(sps[:, 0:w], k_he[:, ksl],
                                     q_he[:, qb * 512 + off:qb * 512 + 512],
                                     start=True, stop=True)
                    et = ep.tile([128, 512], BF, tag="et", name=f"et_{b}_{e}_{qb}_{kt}")
                    if off > 0:
                        nc.vector.memset(et[:, 0:off], 0.0)
                    nc.scalar.activation(et[:, off:512], sps[:, 0:w],
                                         mybir.ActivationFunctionType.Exp, scale=SCALE)
                    if r >= 0:
                        nc.vector.tensor_mul(et[:, off:512], et[:, off:512],
                                             mk_sb[:, r, off:512])
                    if kt == 0:
                        nc.vector.tensor_copy(acc[:], et[:])
                    else:
                        nc.vector.tensor_add(acc[:, off:512], acc[:, off:512],
                                             et[:, off:512])
                    nc.tensor.matmul(yps[:], v_sb[:, kt, e * 128:e * 128 + 128],
                                     et[:], start=(kt == 0), stop=(kt == nkt - 1))
                    if kt == 1 and fstate["pend"] is not None:
                        finalize(*fstate["pend"])
                        fstate["pend"] = None
                if fstate["pend"] is not None:
                    finalize(*fstate["pend"])
                fstate["pend"] = (acc, yps, e, qb)

            units = [lambda qb=qb, e=e: unit(qb, e) for qb in range(4) for e in range(2)]

            def tail():
                finalize(*fstate["pend"])
                nc.gpsimd.collective_compute(
                    "AllToAll", mybir.AluOpType.bypass,
                    ins=[a2a_ins[b][:]], outs=[a2a_outs[b][:]],
                    replica_groups=[list(range(NCORES))],
                )
            return units, tail

        def emit_yres_load(y_res, b):
            for j in range(8):
                for e in range(2):
                    nc.gpsimd.dma_start(out=y_res[:, 2 * j + e, b * 256:b * 256 + 256],
                                        in_=a2a_outs[b][j, e])

        def outproj_db(y_res, tag, db, i_list, eng=None):
            owt = owp.tile([128, 16, 256], BF, tag="ow", name=f"owt_{tag}_{db}")
            (eng or nc.sync).dma_start(out=owt[:], in_=outwT_e[db])
            dsl = slice(db * 256, db * 256 + 256)
            for i in i_list:
                pso = psA.tile([128, 256], F32, tag="ps", name=f"pso_{tag}_{db}_{i}")
                for ft in range(16):
                    nc.tensor.matmul(pso[:], y_res[:, ft, i * 128:i * 128 + 128],
                                     owt[:, ft, :], start=(ft == 0), stop=(ft == 15))
                oev = oep.tile([128, 256], F32, tag="oe", name=f"oev_{tag}_{db}_{i}")
                nc.vector.tensor_copy(oev[:], pso[:])
                nc.sync.dma_start(out=out_e[i * 128:i * 128 + 128, dsl], in_=oev[:])

        def interleave(units, groups, tail):
            """alternate unit/group; leftovers appended; tail last"""
            ui, gi = 0, 0
            while ui < len(units) or gi < len(groups):
                if ui < len(units):
                    units[ui](); ui += 1
                if gi < len(groups):
                    groups[gi](); gi += 1
            tail()

        # ---------- pipeline (sequential per batch) ----------
        x_sb = emit_x_load(0)
        st = {"rots": [], "v": None}
        qk0, v0 = make_qkv_groups(0, x_sb, st)
        for g in qk0:
            g()
        vgroups = v0
        for b in range(B):
            vgroups[0]()  # v quarter 0 before the first unit
            units, tail = make_attn_units(b, st)
            for i, u in enumerate(units):
                u()
                # unit (qb, e): after both units of qb, emit v quarter qb+1
                if i in (1, 3, 5) and (i // 2 + 1) < 4:
                    vgroups[i // 2 + 1]()
            tail()
            if b < B - 1:
                xn = emit_x_load(b + 1)
                st_next = {"rots": [], "v": None}
                qkn, vn = make_qkv_groups(b + 1, xn, st_next)
                for g in qkn:
                    g()
                vgroups = vn
                st = st_next

        # ---- output projection (batch-3 token tiles last, hiding the final A2A) ----
        y_res = big.tile([128, 16, 1024], BF, tag="bigbuf", name="y_res")
        for b in range(B):
            emit_yres_load(y_res, b)
        for db in range(8):
            outproj_db(y_res, "a", db, [0, 1, 2, 3, 4, 5])
        for db in range(8):
            outproj_db(y_res, "b", db, [6, 7])

    nc.compile()
    return nc


def _host_prep(x, qkv_w, out_w):
    """Build the per-core input maps (bf16, pre-transposed/permuted)."""
    import ml_dtypes
    bf16 = ml_dtypes.bfloat16

    # x_pre[b, tb, p, d, s] = x[b, tb*512+s, d*128+p]
    xT = np.ascontiguousarray(
        x.reshape(B, 4, 512, 16, 128).transpose(0, 1, 4, 3, 2)).astype(bf16)
    # outw_pre[db, p, ft, n] = out_w.T[ft*128+p, db*256+n]
    outwT = np.ascontiguousarray(
        out_w.T.reshape(16, 128, 8, 256).transpose(2, 1, 0, 3)).astype(bf16)

    even = np.arange(0, DH, 2)
    odd = np.arange(1, DH, 2)
    freqs = 1.0 / (10000.0 ** (np.arange(0, DH, 2, dtype=np.float64) / DH))
    ang = np.arange(S, dtype=np.float64)[None, :] * freqs[:, None]   # [64, S]
    cs = np.concatenate([np.cos(ang), np.cos(ang)], 0).astype(bf16)  # [128, S]
    sn = np.concatenate([np.sin(ang), np.sin(ang)], 0).astype(bf16)

    masks = np.zeros((4, 128, 512), np.float32)
    for r in range(4):
        for t in range(128):
            masks[r, t, r * 128 + t:] = 1.0
    masks = np.ascontiguousarray(masks.transpose(1, 0, 2)).astype(bf16)  # [128, 4, 512]

    in_maps = []
    for c in range(NCORES):
        h0, h1 = 2 * c, 2 * c + 1
        qA = np.concatenate([h0 * DH + even, h1 * DH + even])
        qB = np.concatenate([h0 * DH + odd, h1 * DH + odd])
        rows_qk = np.concatenate([qA, qB, 2048 + qA, 2048 + qB])
        # wqk_pre[p, d, f] = qkv_w[rows_qk[f], d*128+p]
        wqkT = np.ascontiguousarray(
            qkv_w[rows_qk].T.reshape(16, 128, 512).transpose(1, 0, 2)).astype(bf16)
        wvT = np.ascontiguousarray(
            qkv_w[4096 + h0 * DH: 4096 + (h1 + 1) * DH].T.reshape(16, 128, 256)
            .transpose(1, 0, 2)).astype(bf16)
        in_maps.append({
            "xT": xT, "wqkT": wqkT, "wvT": wvT, "outwT": outwT,
            "cs": cs, "sn": sn, "masks": masks,
        })
    return in_maps


def _ensure_profile_hook():
    """The agent image's antenv lacks axon_hooks; recreate it so that
    run_bass_kernel_spmd(trace=True) (or BASS_TRACE=1) does not crash."""
    import sys, types
    try:
        import antenv.axon_hooks  # noqa
        return
    except ImportError:
        pass
    try:
        from trn_agent_boot.trn_boot import _ntff_profile_via_ctypes
        hook = _ntff_profile_via_ctypes("/opt/axon/libaxon_pjrt.so")
    except Exception:
        hook = None
    mod = types.ModuleType("antenv.axon_hooks")
    mod.get_axon_ntff_profile_hook = lambda: hook

    def set_axon_ntff_profile_hook(h):
        mod.get_axon_ntff_profile_hook = lambda: h

    mod.set_axon_ntff_profile_hook = set_axon_ntff_profile_hook
    sys.modules["antenv.axon_hooks"] = mod
    try:
        import antenv
        antenv.axon_hooks = mod
    except ImportError:
        pass


def kernel(x, qkv_w, qkv_b, out_w, out_b):
    global LAST_RESULT
    from concourse.bass_utils import run_bass_kernel_spmd
    _ensure_profile_hook()

    if "nc" not in _CACHE:
        _CACHE["nc"] = _build_nc()
    nc = _CACHE["nc"]

    in_maps = _host_prep(np.asarray(x, np.float32), np.asarray(qkv_w, np.float32),
                         np.asarray(out_w, np.float32))
    trace = bool(os.environ.get("BASS_KERNEL_TRACE"))
    r = run_bass_kernel_spmd(nc, in_maps, list(range(NCORES)), trace=trace)
    LAST_RESULT = r

    out = np.empty((B, S, D), np.float32)
    for c in range(NCORES):
        shard = r.results[c]["out"]
        for b in range(B):
            out[b, c * 256:(c + 1) * 256, :] = shard[b * 256:(b + 1) * 256]
    return out
